# revision 25
# baseline (speedup 1.0000x reference)
"""Trainium2 Bass kernel for nn_DFMAtt: deformable-flow attention.

Per sample (1x1-conv proj, K=4 flow fields, softmax weights, bilinear
grid-sample of proj at flow-displaced positions, weighted sum over K).

Strategy (one batch sample per NeuronCore, 8 cores data-parallel):
  Flows are tiny, so every bilinear corner lies in a fixed 5x5 window
  dy,dx in [-2,2] around its output pixel.  The whole gather-and-blend
  becomes out = proj @ A with A banded (25 diagonals).  Pipeline
  (software-pipelined so all engines overlap):
    - fused [proj | flows | logits] matmul per 128-position tile (f=268),
      bias folded into the PSUM->SBUF copy (DVE tensor_tensor add),
    - fp16 corner-weight planes on DVE (scalar_tensor_tensor fusions),
      softmax normalization folded into e^logits, two half-size batches,
    - partition-shift into source-index space via TensorE rotation
      matmuls against identity slices (PSUM), NOT per-partition DMAs,
    - per-pair banded blocks A [128 x 2*578] via gpsimd.local_scatter;
      border validity is baked into the per-tile scatter indices as -1,
    - main contraction on TensorE fp16, fp16 output.
"""

import os
import sys

sys.path.insert(0, "/opt/trn_rl_repo")

import numpy as np

import concourse.bass as bass
import concourse.mybir as mybir
from concourse import bacc
from concourse.bass import ts
from concourse.tile import TileContext

H = W = 96
C = 256
O = 256
K = 4
N = H * W            # 9216
NT = N // 128        # 72 position tiles
ALPHA = float(W) / float(W - 1)
DYS = list(range(-2, 3))   # -2..2
DXS = list(range(-2, 3))   # -2..2
SHIFTS = [(dy, dx) for dy in DYS for dx in DXS]
NS = len(SHIFTS)     # 25
NSP = 26             # padded (local_scatter needs even num_idxs)
WOFF = 256           # A_r covers n in [r*128 - WOFF, r*128 - WOFF + AW)
AW = 578             # window width; j = q + WOFF - delta_s in [62, 578)
                     # (AW > 516 so r=4b+2 fully covers block b -> single
                     # start=True per PSUM accumulation group)
NBLK = N // 512      # 18 output column blocks
NPAIR = NT // 2      # 36 scatter pairs (2 tiles per local_scatter)
AGRP = [4, 14, 18, 18, 18]     # fused-matmul groups (x-DMA granularity)
BHALF = [(0, 36), (36, 72)]    # plane-pipeline batches
CBATCH = [(0, 34), (34, 72)]   # rotation batches (need planes_n <= t1+2)
SCHUNK = [(0, 13), (13, NS)]   # rotation PSUM s-splits (<=2KB/bank)
APAD = 0                       # (pad-skip removed: it coupled scatters to DVE)
GUARD = 2                      # zero guard tiles each side of planes_n
FUSED = O + 3 * K    # 268 = proj | fx | fy | logits

F32 = mybir.dt.float32
F16 = mybir.dt.float16
I16 = mybir.dt.int16
I32 = mybir.dt.int32
OP = mybir.AluOpType
AF = mybir.ActivationFunctionType


def _host_consts(Wc, bc, Woff, boff, Wwt, bwt):
    """Host-side constant tensors baked into the NEFF."""
    # fused weight matrix [256, 268]: [Wc^T | a*Woff_x | a*Woff_y | Wwt^T]
    wf = np.concatenate(
        [
            Wc.T.astype(np.float32),                       # [c, 256]
            (ALPHA * Woff[:, 0, :]).T.astype(np.float32),  # [c, 4] fx_k
            (ALPHA * Woff[:, 1, :]).T.astype(np.float32),  # [c, 4] fy_k
            Wwt.T.astype(np.float32),                      # [c, 4]
        ],
        axis=1,
    ).astype(np.float16)
    pbias = np.concatenate([bc.astype(np.float32),
                            np.zeros(3 * K, np.float32)]).astype(np.float16)
    biasbc = np.broadcast_to(pbias[None, :], (128, FUSED)).copy()
    biasrow = pbias[None, :].copy()                    # [1, 268]
    ones = np.ones((1, 128), dtype=np.float16)

    # position fields: n = t*128 + p  ->  F[p, t]; d = ix - gx = fields_x + (a-1)gx
    n_grid = np.arange(N, dtype=np.int64).reshape(NT, 128).T   # [128, 72]
    gx = (n_grid % W).astype(np.float64)
    gy = (n_grid // W).astype(np.float64)

    def rep4(f):  # [128, 72] -> [128, 72, 4]
        return np.repeat(f[:, :, None].astype(np.float32), 4, axis=2)

    # +4.0 biases d into (1, 7) so float->int truncation == floor; the
    # flow-field biases (a*boff - 0.5) fold in per-k.  fp32: fp16 ulp at
    # ~5 is 4e-3, too coarse for bilinear weights.
    dgx4 = rep4((ALPHA - 1.0) * gx + 4.0) + (ALPHA * boff[:, 0] - 0.5
                                             ).astype(np.float32)[None, None, :]
    dgy4 = rep4((ALPHA - 1.0) * gy + 4.0) + (ALPHA * boff[:, 1] - 0.5
                                             ).astype(np.float32)[None, None, :]
    # softmax logit bias as a multiplicative e^bwt factor
    ek4 = np.broadcast_to(np.exp(bwt).astype(np.float16)[None, None, :],
                          (128, NT, 4)).copy()

    # rotation operator bank [0_128 | I | 0_128]: column slices give the
    # shifted identities for both rotation pieces (see stage_C)
    dop = np.zeros((128, 384), dtype=np.float16)
    dop[:, 128:256] = np.eye(128, dtype=np.float16)

    # scatter indices per tile pair, with x-wrap / n-range validity as -1.
    # pair p covers r = 2p (cols 0..AW-1) and r = 2p+1 (cols AW..2AW-1).
    deltas = np.array([dy * W + dx for dy, dx in SHIFTS], dtype=np.int64)
    idxp = np.full((128, NPAIR, 2 * NSP), -1, dtype=np.int16)
    for p in range(NPAIR):
        for half in range(2):
            r = 2 * p + half
            for s, (dy, dx) in enumerate(SHIFTS):
                d = deltas[s]
                for q in range(128):
                    n = r * 128 + q - d          # source output position
                    if n < 0 or n >= N:
                        continue                 # never read (col clipped)
                    if not (0 <= (n % W) + dx <= W - 1):
                        continue                 # x-wrap invalid tap
                    j = q + WOFF - d
                    assert APAD <= j < AW
                    idxp[q, p, half * NSP + s] = j - APAD + half * (AW - APAD)
    return wf, biasbc, biasrow, ones, dgx4, dgy4, ek4, dop, idxp


def build_program(Wc, bc, Woff, boff, Wwt, bwt):
    wf_np, biasbc_np, biasrow_np, ones_np, dgx4_np, dgy4_np, ek4_np, dop_np, idxp_np = _host_consts(
        Wc, bc, Woff, boff, Wwt, bwt)

    nc = bacc.Bacc()
    x_in = nc.dram_tensor("x", [C, N], F16, kind="ExternalInput")
    out_d = nc.dram_tensor("out", [O, N], F16, kind="ExternalOutput")

    wf_d = nc.inline_tensor(wf_np, "wf_c")
    biasbc_d = nc.inline_tensor(biasbc_np, "biasbc_c")
    biasrow_d = nc.inline_tensor(biasrow_np, "biasrow_c")
    ones_d = nc.inline_tensor(ones_np, "ones_c")
    ek4_d = nc.inline_tensor(ek4_np, "ek4_c")
    dgx4_d = nc.inline_tensor(dgx4_np, "dgx4_c")
    dgy4_d = nc.inline_tensor(dgy4_np, "dgy4_c")
    dop_d = nc.inline_tensor(dop_np, "dop_c")
    idxp_d = nc.inline_tensor(idxp_np, "idxp_c")

    agst = []
    t0 = 0
    for gsz in AGRP:
        agst.append(t0)
        t0 += gsz

    with TileContext(nc) as tc, nc.allow_low_precision(reason="f16 bilinear weights"):
        with (
            tc.tile_pool(name="consts", bufs=1) as cpool,
            tc.tile_pool(name="big", bufs=1) as big,
            tc.tile_pool(name="apool", bufs=14) as apool,
            tc.tile_pool(name="work", bufs=2) as wpool,
            tc.tile_pool(name="opool", bufs=4) as opool,
            tc.tile_pool(name="ppsum", bufs=3, space="PSUM") as ppsum,
            tc.tile_pool(name="opsum", bufs=2, space="PSUM") as opsum,
            tc.tile_pool(name="shpsum", bufs=2, space="PSUM") as shpsum,
        ):
            # ---- constants + input, interleaved across both HW DGE queues
            # so stage_A can start ~3us in: weights first, then x chunks
            # (halves split sync/scalar), bulky late-use consts last.
            wf = cpool.tile([128, 2, FUSED], F16, tag="wf")
            nc.sync.dma_start(out=wf[:, 0], in_=wf_d[0:128, :])
            nc.sync.dma_start(out=wf[:, 1], in_=wf_d[128:256, :])
            biasbc = cpool.tile([128, FUSED], F16, tag="biasbc")
            nc.scalar.dma_start(out=biasbc[:], in_=biasbc_d[:])
            biasrow = cpool.tile([1, FUSED], F16, tag="biasrow")
            nc.scalar.dma_start(out=biasrow[:], in_=biasrow_d[:])
            ones_sb = cpool.tile([1, 128], F16, tag="ones_sb")
            nc.scalar.dma_start(out=ones_sb[:], in_=ones_d[:])
            ek4 = cpool.tile([128, NT, 4], F16, tag="ek4")
            nc.scalar.dma_start(out=ek4[:], in_=ek4_d[:])
            xg = []
            for g, gsz in enumerate(AGRP):
                xt = big.tile([128, 2, gsz * 128], F16, tag=f"xg{g}", name=f"xg{g}")
                xg.append(xt)
            dgx4 = cpool.tile([128, NT, 4], F32, tag="dgx4")
            dgy4 = cpool.tile([128, NT, 4], F32, tag="dgy4")
            dop = cpool.tile([128, 384], F16, tag="dop")
            idxp = cpool.tile([128, NPAIR, 2 * NSP], I16, tag="idxp")

            def xdma(g):
                c0 = agst[g] * 128
                c1 = c0 + AGRP[g] * 128
                nc.sync.dma_start(out=xg[g][:, 0], in_=x_in[0:128, c0:c1])
                nc.scalar.dma_start(out=xg[g][:, 1], in_=x_in[128:256, c0:c1])

            xdma(0)
            xdma(1)
            nc.sync.dma_start(out=dgx4[:], in_=dgx4_d[:])
            nc.scalar.dma_start(out=dgy4[:], in_=dgy4_d[:])
            xdma(2)
            xdma(3)
            nc.sync.dma_start(out=dop[:], in_=dop_d[:])
            nc.scalar.dma_start(out=idxp[:], in_=idxp_d[:])
            for g in range(4, len(AGRP)):
                xdma(g)

            pfbuf = big.tile([128, NT, FUSED], F16, tag="pfbuf")
            # planes_n with GUARD zero tiles each side (rotation halo)
            planes_ng = big.tile([128, NS, NT + 2 * GUARD], F16, tag="planes_ng")
            planes_m = big.tile([128, NS, NT], F16, tag="planes_m")
            mp = big.tile([128, NT, NSP], F16, tag="mp")
            nc.vector.memset(planes_ng[:, :, 0:GUARD], 0.0)
            nc.vector.memset(planes_ng[:, :, GUARD + NT:], 0.0)
            nc.vector.memset(mp[:, :, NS:], 0.0)

            # ---------- pipeline stages ----------
            def stage_A(g):
                """Fused [proj|fields] matmuls for group g -> pfbuf (fp16)."""
                for i in range(AGRP[g]):
                    t = agst[g] + i
                    pp = ppsum.tile([128, FUSED], F32, tag="pp")
                    nc.tensor.matmul(pp[:], xg[g][:, 0, ts(i, 128)], wf[:, 0, :],
                                     start=True, stop=False)
                    nc.tensor.matmul(pp[:], xg[g][:, 1, ts(i, 128)], wf[:, 1, :],
                                     start=False, stop=(t % 2 == 0))
                    if t % 2 == 0:
                        # proj bias folded into the DVE copy
                        nc.vector.tensor_add(out=pfbuf[:, t, :], in0=pp[:],
                                             in1=biasbc[:])
                    else:
                        # proj bias via rank-1 matmul; plain copy on Act
                        nc.tensor.matmul(pp[:, 0:O], ones_sb[:],
                                         biasrow[:, 0:O], start=False, stop=True)
                        nc.scalar.activation(pfbuf[:, t, :], pp[:], AF.Copy)

            def stage_B(h):
                """Corner-weight planes for tile batch h -> planes_ng.

                Work tiles are flat [128, gsz*4]: dense elementwise ops use
                2D APs (cheaper DVE issue); only pfbuf reads, the k-reduce
                and quad products need 3D views.
                """
                a, b = BHALF[h]
                gsz = b - a
                shpf = [128, gsz * 4]

                def t3(t):  # [128, gsz*4] -> [128, gsz, 4] view
                    return t.rearrange("p (t k) -> p t k", k=4)

                fx = pfbuf[:, a:b, O:O + 4]
                fy = pfbuf[:, a:b, O + 4:O + 8]
                lg = pfbuf[:, a:b, O + 8:O + 12]

                d_x = wpool.tile(shpf, F32, tag="d_x", name="d_x")
                d_y = wpool.tile(shpf, F32, tag="d_y", name="d_y")
                nc.vector.tensor_add(out=t3(d_x[:]), in0=fx, in1=dgx4[:, a:b, :])
                nc.vector.tensor_add(out=t3(d_y[:]), in0=fy, in1=dgy4[:, a:b, :])

                def floor4(src_, tag):
                    # int cast may round on HW; is_gt correction makes floor
                    ii = wpool.tile(shpf, I32, tag=f"{tag}i", name=f"{tag}i")
                    rf = wpool.tile(shpf, F32, tag=f"{tag}r", name=f"{tag}r")
                    gt = wpool.tile(shpf, F32, tag=f"{tag}g", name=f"{tag}g")
                    x0 = wpool.tile(shpf, F32, tag=f"{tag}0", name=f"{tag}0")
                    nc.vector.tensor_copy(out=ii[:], in_=src_[:])
                    nc.vector.tensor_copy(out=rf[:], in_=ii[:])
                    nc.vector.tensor_tensor(out=gt[:], in0=rf[:], in1=src_[:],
                                            op=OP.is_gt)
                    nc.vector.tensor_sub(out=x0[:], in0=rf[:], in1=gt[:])
                    # clamp offset-floor to taps [-2, 1]: extrapolate rare
                    # out-of-band corners instead of dropping them
                    nc.vector.tensor_scalar(out=x0[:], in0=x0[:], scalar1=2.0,
                                            scalar2=5.0, op0=OP.max, op1=OP.min)
                    return x0

                x0f = floor4(d_x, "fx")
                y0f = floor4(d_y, "fy")

                wx1 = wpool.tile(shpf, F16, tag="wx1", name="wx1")
                wy1 = wpool.tile(shpf, F16, tag="wy1", name="wy1")
                wx0 = wpool.tile(shpf, F16, tag="wx0", name="wx0")
                wy0 = wpool.tile(shpf, F16, tag="wy0", name="wy0")
                nc.vector.tensor_sub(out=wx1[:], in0=d_x[:], in1=x0f[:])
                nc.vector.tensor_sub(out=wy1[:], in0=d_y[:], in1=y0f[:])
                nc.vector.tensor_scalar(out=wx0[:], in0=wx1[:], scalar1=-1.0,
                                        scalar2=1.0, op0=OP.mult, op1=OP.add)
                nc.vector.tensor_scalar(out=wy0[:], in0=wy1[:], scalar1=-1.0,
                                        scalar2=1.0, op0=OP.mult, op1=OP.add)

                # softmax numerators; logit bias enters as the e^bwt factor
                e4r = wpool.tile(shpf, F16, tag="e4r", name="e4r")
                nc.scalar.activation(t3(e4r[:]), lg, AF.Exp)
                e4 = wpool.tile(shpf, F16, tag="e4", name="e4")
                nc.vector.tensor_mul(out=t3(e4[:]), in0=t3(e4r[:]),
                                     in1=ek4[:, a:b, :])
                ssum = wpool.tile([128, gsz], F32, tag="ssum", name="ssum")
                nc.vector.tensor_reduce(out=ssum[:], in_=t3(e4[:]),
                                        axis=mybir.AxisListType.X, op=OP.add)
                recb = wpool.tile(shpf, F16, tag="recb", name="recb")
                for k in range(4):
                    nc.vector.reciprocal(t3(recb[:])[:, :, k], ssum[:])
                e4n = wpool.tile(shpf, F16, tag="e4n", name="e4n")
                nc.vector.tensor_mul(out=e4n[:], in0=e4[:], in1=recb[:])
                wy1e = wpool.tile(shpf, F16, tag="wy1e", name="wy1e")
                wy0e = wpool.tile(shpf, F16, tag="wy0e", name="wy0e")
                nc.vector.tensor_mul(out=wy1e[:], in0=wy1[:], in1=e4n[:])
                nc.vector.tensor_mul(out=wy0e[:], in0=wy0[:], in1=e4n[:])

                def taps(x0, w0t, w1t, tag):
                    # tp[v] = (x0==v+4)*w0 + (x0==v+3)*w1 for v in -2..2
                    tp = {}
                    tmp = wpool.tile(shpf, F16, tag=f"{tag}tmp", name=f"{tag}tmp")
                    for v in DXS:
                        h = wpool.tile(shpf, F16, tag=f"{tag}{v}", name=f"{tag}{v}")
                        if v == -2:
                            nc.vector.scalar_tensor_tensor(
                                out=h[:], in0=x0[:], scalar=2.0, in1=w0t[:],
                                op0=OP.is_equal, op1=OP.mult)
                        elif v == 2:
                            nc.vector.scalar_tensor_tensor(
                                out=h[:], in0=x0[:], scalar=5.0, in1=w1t[:],
                                op0=OP.is_equal, op1=OP.mult)
                        else:
                            nc.vector.scalar_tensor_tensor(
                                out=h[:], in0=x0[:], scalar=float(v + 4),
                                in1=w0t[:], op0=OP.is_equal, op1=OP.mult)
                            nc.vector.scalar_tensor_tensor(
                                out=tmp[:], in0=x0[:], scalar=float(v + 3),
                                in1=w1t[:], op0=OP.is_equal, op1=OP.mult)
                            nc.vector.tensor_add(out=h[:], in0=h[:], in1=tmp[:])
                        tp[v] = h
                    return tp

                hx = taps(x0f, wx0, wx1, "hx")
                vy = taps(y0f, wy0e, wy1e, "vy")

                # quad-batched products: 4 s-planes share one X-reduce
                prodq = wpool.tile([128, gsz, 4, 4], F16, tag="prodq",
                                   name="prodq")
                for s0 in range(0, NS, 4):
                    s1 = min(NS, s0 + 4)
                    for s in range(s0, s1):
                        dyv, dxv = SHIFTS[s]
                        nc.vector.tensor_mul(out=prodq[:, :, s - s0, :],
                                             in0=t3(vy[dyv][:]),
                                             in1=t3(hx[dxv][:]))
                    nc.vector.tensor_reduce(
                        out=planes_ng[:, s0:s1, GUARD + a:GUARD + b]
                            .transpose([0, 2, 1]),
                        in_=prodq[:, :, 0:s1 - s0, :],
                        axis=mybir.AxisListType.X, op=OP.add)

            def stage_C(ci):
                """Partition-rotation n->m via TensorE for batch ci."""
                t0c, t1c = CBATCH[ci]
                tb = t1c - t0c
                for si, (s0, s1) in enumerate(SCHUNK):
                    ps = shpsum.tile([128, 13, tb], F32, tag="sh",
                                     name="sh", bufs=1)
                    for s in range(s0, s1):
                        dyv, dxv = SHIFTS[s]
                        delta = dyv * W + dxv
                        b = delta % 128
                        a = (delta - b) // 128
                        # piece 1: rows q>=b <- planes_n[q-b, t-a]; rest 0
                        nc.tensor.matmul(
                            ps[:, s - s0, :],
                            dop[:, 128 - b:256 - b],
                            planes_ng[:, s, GUARD + t0c - a:GUARD + t1c - a],
                            start=True, stop=(b == 0))
                        # piece 2: rows q<b += planes_n[128-b+q, t-a-1]
                        if b > 0:
                            nc.tensor.matmul(
                                ps[:, s - s0, :],
                                dop[:, 256 - b:384 - b],
                                planes_ng[:, s,
                                          GUARD + t0c - a - 1:GUARD + t1c - a - 1],
                                start=False, stop=True)
                    nc.scalar.activation(planes_m[:, s0:s1, t0c:t1c],
                                         ps[:, 0:s1 - s0, :], AF.Copy)

            a_pairs = [None] * NPAIR

            def repack(p0, p1):
                """mp[:, t, s] <- planes_m[:, s, t] for pairs [p0, p1)."""
                nc.gpsimd.tensor_copy(
                    out=mp[:, 2 * p0:2 * p1, 0:NS],
                    in_=planes_m[:, 0:NS, 2 * p0:2 * p1].transpose([0, 2, 1]),
                )

            def scatter(p):
                at = apool.tile([128, 2 * AW], F16, tag="a")
                nc.gpsimd.local_scatter(at[:], mp[:, 2 * p:2 * p + 2, :],
                                        idxp[:, p, :], channels=128,
                                        num_elems=2 * AW, num_idxs=2 * NSP)
                a_pairs[p] = at

            def stage_E(p0, p1):
                for c0 in range(p0, p1, 4):
                    repack(c0, min(p1, c0 + 4))
                    for p in range(c0, min(p1, c0 + 4)):
                        scatter(p)

            def a_cols(r, j0, j1):
                # even r at buffer cols [0, AW) (j-aligned, [0, APAD) zero);
                # odd r data at [AW, 2*AW-APAD) holding j in [APAD, AW)
                at = a_pairs[r // 2]
                off = (r % 2) * (AW - APAD)
                return at[:, off + j0:off + j1]

            def stage_F(b):
                """Main contraction for output block b, o-halves interleaved
                across two PSUM banks to hide accumulation-chain latency."""
                B = 512 * b
                rs = list(range(max(0, 4 * b - 2), min(NT, 4 * b + 6)))
                r_full = 4 * b + 2           # window [B, B+578) covers the block
                prog = [(r_full, B, B + 512)]
                for r in rs:
                    if r == r_full:
                        continue
                    w0 = 128 * r - WOFF
                    n0, n1 = max(B, w0 + APAD), min(B + 512, w0 + AW)
                    if n1 > n0:
                        prog.append((r, n0, n1))
                po = [opsum.tile([128, 512], F32, tag=f"po{oh}", name=f"po{oh}")
                      for oh in range(2)]
                for i, (r, n0, n1) in enumerate(prog):
                    w0 = 128 * r - WOFF
                    for oh in range(2):
                        nc.tensor.matmul(
                            po[oh][:, n0 - B:n1 - B],
                            pfbuf[:, r, ts(oh, 128)],
                            a_cols(r, n0 - w0, n1 - w0),
                            start=(i == 0),
                            stop=(i == len(prog) - 1),
                        )
                for oh in range(2):
                    ob = opool.tile([128, 512], F16, tag="ob", name="ob")
                    nc.scalar.activation(ob[:], po[oh][:], AF.Copy)
                    nc.sync.dma_start(out=out_d[ts(oh, 128), ts(b, 512)],
                                      in_=ob[:])

            # ---------- schedule ----------
            stage_A(0)
            stage_A(1)
            stage_A(2)
            stage_B(0)          # tiles [0, 36): needs pfbuf <= 35 (A0-A2)
            stage_A(3)
            stage_C(0)          # rotation for tiles [0, 34)
            stage_E(0, 17)      # pairs 0-16 (tiles 0-33)
            stage_A(4)
            stage_B(1)          # tiles [36, 72)
            for b in range(0, 3):
                stage_F(b)      # pairs <= 6
            stage_C(1)          # rotation for tiles [34, 72)
            stage_E(17, NPAIR)  # pairs 17-35
            for b in range(3, NBLK):
                stage_F(b)      # F3-F7 pairs <= 16; F8+ from E2
    nc.finalize()
    return nc


_CACHE = {}


def _get_program(inputs):
    key = "prog"
    if key not in _CACHE:
        _CACHE[key] = build_program(
            np.asarray(inputs["Wc"], np.float32),
            np.asarray(inputs["bc"], np.float32),
            np.asarray(inputs["Woff"], np.float32),
            np.asarray(inputs["boff"], np.float32),
            np.asarray(inputs["Wwt"], np.float32),
            np.asarray(inputs["bwt"], np.float32),
        )
    return _CACHE[key]


def kernel(x, Wc, bc, Woff, boff, Wwt, bwt, _trace=False):
    from concourse.bass_utils import run_bass_kernel_spmd

    x = np.asarray(x, np.float32)
    b = x.shape[0]
    assert x.shape == (b, C, H, W) and b == 8

    nc = _get_program(dict(Wc=Wc, bc=bc, Woff=Woff, boff=boff, Wwt=Wwt, bwt=bwt))
    in_maps = [
        {"x": np.ascontiguousarray(x[i].reshape(C, N).astype(np.float16))}
        for i in range(b)
    ]
    res = run_bass_kernel_spmd(nc, in_maps, core_ids=list(range(b)), trace=_trace)
    _CACHE["last_results"] = res
    out = np.stack([res.results[i]["out"].reshape(O, H, W) for i in range(b)])
    return out.astype(np.float32)


# revision 26
# speedup vs baseline: 1.0768x; 1.0768x over previous
"""Trainium2 Bass kernel for nn_DFMAtt: deformable-flow attention.

Per sample (1x1-conv proj, K=4 flow fields, softmax weights, bilinear
grid-sample of proj at flow-displaced positions, weighted sum over K).

Strategy (one batch sample per NeuronCore, 8 cores data-parallel):
  Flows are tiny, so every bilinear corner lies in a fixed 5x5 window
  dy,dx in [-2,2] around its output pixel.  The whole gather-and-blend
  becomes out = proj @ A with A banded (25 diagonals).  Pipeline
  (software-pipelined so all engines overlap):
    - fused [proj | flows | logits] matmul per 128-position tile (f=268),
      bias folded into the PSUM->SBUF copy (DVE tensor_tensor add),
    - fp16 corner-weight planes on DVE (scalar_tensor_tensor fusions),
      softmax normalization folded into e^logits, two half-size batches,
    - partition-shift into source-index space via TensorE rotation
      matmuls against identity slices (PSUM), NOT per-partition DMAs,
    - per-pair banded blocks A [128 x 2*578] via gpsimd.local_scatter;
      border validity is baked into the per-tile scatter indices as -1,
    - main contraction on TensorE fp16, fp16 output.
"""

import os
import sys

sys.path.insert(0, "/opt/trn_rl_repo")

import numpy as np

import concourse.bass as bass
import concourse.mybir as mybir
from concourse import bacc
from concourse.bass import ts
from concourse.tile import TileContext

H = W = 96
C = 256
O = 256
K = 4
N = H * W            # 9216
NT = N // 128        # 72 position tiles
ALPHA = float(W) / float(W - 1)
DYS = list(range(-2, 3))   # -2..2
DXS = list(range(-2, 3))   # -2..2
SHIFTS = [(dy, dx) for dy in DYS for dx in DXS]
NS = len(SHIFTS)     # 25
NSP = 26             # padded (local_scatter needs even num_idxs)
WOFF = 256           # A_r covers n in [r*128 - WOFF, r*128 - WOFF + AW)
AW = 578             # window width; j = q + WOFF - delta_s in [62, 578)
                     # (AW > 516 so r=4b+2 fully covers block b -> single
                     # start=True per PSUM accumulation group)
NBLK = N // 512      # 18 output column blocks
NPAIR = NT // 2      # 36 scatter pairs (2 tiles per local_scatter)
AGRP = [4, 14, 18, 18, 18]     # fused-matmul groups (x-DMA granularity)
BHALF = [(0, 36), (36, 72)]    # plane-pipeline batches
CBATCH = [(0, 34), (34, 72)]   # rotation batches (need planes_n <= t1+2)
SCHUNK = [(0, 13), (13, NS)]   # rotation PSUM s-splits (<=2KB/bank)
APAD = 0                       # (pad-skip removed: it coupled scatters to DVE)
GUARD = 2                      # zero guard tiles each side of planes_n
FUSED = O + 3 * K    # 268 = proj | fx | fy | logits

F32 = mybir.dt.float32
F16 = mybir.dt.float16
I16 = mybir.dt.int16
I32 = mybir.dt.int32
OP = mybir.AluOpType
AF = mybir.ActivationFunctionType


def _host_consts(Wc, bc, Woff, boff, Wwt, bwt):
    """Host-side constant tensors baked into the NEFF."""
    # fused weight matrix [256, 268]: [Wc^T | a*Woff_x | a*Woff_y | Wwt^T]
    wf = np.concatenate(
        [
            Wc.T.astype(np.float32),                       # [c, 256]
            (ALPHA * Woff[:, 0, :]).T.astype(np.float32),  # [c, 4] fx_k
            (ALPHA * Woff[:, 1, :]).T.astype(np.float32),  # [c, 4] fy_k
            Wwt.T.astype(np.float32),                      # [c, 4]
        ],
        axis=1,
    ).astype(np.float16)
    pbias = np.concatenate([bc.astype(np.float32),
                            np.zeros(3 * K, np.float32)]).astype(np.float16)
    biasbc = np.broadcast_to(pbias[None, :], (128, FUSED)).copy()
    biasrow = pbias[None, :].copy()                    # [1, 268]
    ones = np.ones((1, 128), dtype=np.float16)

    # position fields: n = t*128 + p  ->  F[p, t]; d = ix - gx = fields_x + (a-1)gx
    n_grid = np.arange(N, dtype=np.int64).reshape(NT, 128).T   # [128, 72]
    gx = (n_grid % W).astype(np.float64)
    gy = (n_grid // W).astype(np.float64)

    def rep4(f):  # [128, 72] -> [128, 72, 4]
        return np.repeat(f[:, :, None].astype(np.float32), 4, axis=2)

    # +4.0 biases d into (1, 7) so float->int truncation == floor; the
    # flow-field biases (a*boff - 0.5) fold in per-k.  fp32: fp16 ulp at
    # ~5 is 4e-3, too coarse for bilinear weights.
    dgx4 = rep4((ALPHA - 1.0) * gx + 4.0) + (ALPHA * boff[:, 0] - 0.5
                                             ).astype(np.float32)[None, None, :]
    dgy4 = rep4((ALPHA - 1.0) * gy + 4.0) + (ALPHA * boff[:, 1] - 0.5
                                             ).astype(np.float32)[None, None, :]
    # softmax logit bias as a multiplicative e^bwt factor
    ek4 = np.broadcast_to(np.exp(bwt).astype(np.float16)[None, None, :],
                          (128, NT, 4)).copy()

    # rotation operator bank [0_128 | I | 0_128]: column slices give the
    # shifted identities for both rotation pieces (see stage_C)
    dop = np.zeros((128, 384), dtype=np.float16)
    dop[:, 128:256] = np.eye(128, dtype=np.float16)

    # scatter indices per tile pair, with x-wrap / n-range validity as -1.
    # pair p covers r = 2p (cols 0..AW-1) and r = 2p+1 (cols AW..2AW-1).
    deltas = np.array([dy * W + dx for dy, dx in SHIFTS], dtype=np.int64)
    idxp = np.full((128, NPAIR, 2 * NSP), -1, dtype=np.int16)
    for p in range(NPAIR):
        for half in range(2):
            r = 2 * p + half
            for s, (dy, dx) in enumerate(SHIFTS):
                d = deltas[s]
                for q in range(128):
                    n = r * 128 + q - d          # source output position
                    if n < 0 or n >= N:
                        continue                 # never read (col clipped)
                    if not (0 <= (n % W) + dx <= W - 1):
                        continue                 # x-wrap invalid tap
                    j = q + WOFF - d
                    assert APAD <= j < AW
                    idxp[q, p, half * NSP + s] = j - APAD + half * (AW - APAD)
    return wf, biasbc, biasrow, ones, dgx4, dgy4, ek4, dop, idxp


def build_program(Wc, bc, Woff, boff, Wwt, bwt):
    wf_np, biasbc_np, biasrow_np, ones_np, dgx4_np, dgy4_np, ek4_np, dop_np, idxp_np = _host_consts(
        Wc, bc, Woff, boff, Wwt, bwt)

    nc = bacc.Bacc()
    x_in = nc.dram_tensor("x", [C, N], F16, kind="ExternalInput")
    out_d = nc.dram_tensor("out", [O, N], F16, kind="ExternalOutput")

    wf_d = nc.inline_tensor(wf_np, "wf_c")
    biasbc_d = nc.inline_tensor(biasbc_np, "biasbc_c")
    biasrow_d = nc.inline_tensor(biasrow_np, "biasrow_c")
    ones_d = nc.inline_tensor(ones_np, "ones_c")
    ek4_d = nc.inline_tensor(ek4_np, "ek4_c")
    dgx4_d = nc.inline_tensor(dgx4_np, "dgx4_c")
    dgy4_d = nc.inline_tensor(dgy4_np, "dgy4_c")
    dop_d = nc.inline_tensor(dop_np, "dop_c")
    idxp_d = nc.inline_tensor(idxp_np, "idxp_c")

    agst = []
    t0 = 0
    for gsz in AGRP:
        agst.append(t0)
        t0 += gsz

    with TileContext(nc) as tc, nc.allow_low_precision(reason="f16 bilinear weights"):
        with (
            tc.tile_pool(name="consts", bufs=1) as cpool,
            tc.tile_pool(name="big", bufs=1) as big,
            tc.tile_pool(name="apool", bufs=14) as apool,
            tc.tile_pool(name="work", bufs=2) as wpool,
            tc.tile_pool(name="opool", bufs=4) as opool,
            tc.tile_pool(name="ppsum", bufs=3, space="PSUM") as ppsum,
            tc.tile_pool(name="opsum", bufs=2, space="PSUM") as opsum,
            tc.tile_pool(name="shpsum", bufs=2, space="PSUM") as shpsum,
        ):
            # ---- constants + input, interleaved across both HW DGE queues
            # so stage_A can start ~3us in: weights first, then x chunks
            # (halves split sync/scalar), bulky late-use consts last.
            wf = cpool.tile([128, 2, FUSED], F16, tag="wf")
            nc.sync.dma_start(out=wf[:, 0], in_=wf_d[0:128, :])
            nc.sync.dma_start(out=wf[:, 1], in_=wf_d[128:256, :])
            biasbc = cpool.tile([128, FUSED], F16, tag="biasbc")
            nc.scalar.dma_start(out=biasbc[:], in_=biasbc_d[:])
            biasrow = cpool.tile([1, FUSED], F16, tag="biasrow")
            nc.scalar.dma_start(out=biasrow[:], in_=biasrow_d[:])
            ones_sb = cpool.tile([1, 128], F16, tag="ones_sb")
            nc.scalar.dma_start(out=ones_sb[:], in_=ones_d[:])
            ek4 = cpool.tile([128, NT, 4], F16, tag="ek4")
            nc.scalar.dma_start(out=ek4[:], in_=ek4_d[:])
            xg = []
            for g, gsz in enumerate(AGRP):
                xt = big.tile([128, 2, gsz * 128], F16, tag=f"xg{g}", name=f"xg{g}")
                xg.append(xt)
            dgx4 = cpool.tile([128, NT, 4], F32, tag="dgx4")
            dgy4 = cpool.tile([128, NT, 4], F32, tag="dgy4")
            dop = cpool.tile([128, 384], F16, tag="dop")
            idxp = cpool.tile([128, NPAIR, 2 * NSP], I16, tag="idxp")

            def xdma(g):
                c0 = agst[g] * 128
                c1 = c0 + AGRP[g] * 128
                nc.sync.dma_start(out=xg[g][:, 0], in_=x_in[0:128, c0:c1])
                nc.scalar.dma_start(out=xg[g][:, 1], in_=x_in[128:256, c0:c1])

            xdma(0)
            xdma(1)
            nc.sync.dma_start(out=dgx4[:], in_=dgx4_d[:])
            nc.scalar.dma_start(out=dgy4[:], in_=dgy4_d[:])
            xdma(2)
            xdma(3)
            nc.sync.dma_start(out=dop[:], in_=dop_d[:])
            nc.scalar.dma_start(out=idxp[:], in_=idxp_d[:])
            for g in range(4, len(AGRP)):
                xdma(g)

            pfbuf = big.tile([128, NT, FUSED], F16, tag="pfbuf")
            # planes_n with GUARD zero tiles each side (rotation halo)
            planes_ng = big.tile([128, NS, NT + 2 * GUARD], F16, tag="planes_ng")
            planes_m = big.tile([128, NS, NT], F16, tag="planes_m")
            mp = big.tile([128, NT, NSP], F16, tag="mp")
            nc.vector.memset(planes_ng[:, :, 0:GUARD], 0.0)
            nc.vector.memset(planes_ng[:, :, GUARD + NT:], 0.0)
            nc.vector.memset(mp[:, :, NS:], 0.0)

            # ---------- pipeline stages ----------
            def stage_A(g):
                """Fused [proj|fields] matmuls for group g -> pfbuf (fp16)."""
                for i in range(AGRP[g]):
                    t = agst[g] + i
                    pp = ppsum.tile([128, FUSED], F32, tag="pp")
                    nc.tensor.matmul(pp[:], xg[g][:, 0, ts(i, 128)], wf[:, 0, :],
                                     start=True, stop=False)
                    nc.tensor.matmul(pp[:], xg[g][:, 1, ts(i, 128)], wf[:, 1, :],
                                     start=False, stop=True)
                    # proj bias folded into the DVE copy
                    nc.vector.tensor_add(out=pfbuf[:, t, :], in0=pp[:],
                                         in1=biasbc[:])

            def stage_B(h):
                """Corner-weight planes for tile batch h -> planes_ng.

                Work tiles are flat [128, gsz*4]: dense elementwise ops use
                2D APs (cheaper DVE issue); only pfbuf reads, the k-reduce
                and quad products need 3D views.
                """
                a, b = BHALF[h]
                gsz = b - a
                shpf = [128, gsz * 4]

                def t3(t):  # [128, gsz*4] -> [128, gsz, 4] view
                    return t.rearrange("p (t k) -> p t k", k=4)

                fx = pfbuf[:, a:b, O:O + 4]
                fy = pfbuf[:, a:b, O + 4:O + 8]
                lg = pfbuf[:, a:b, O + 8:O + 12]

                d_x = wpool.tile(shpf, F32, tag="d_x", name="d_x")
                d_y = wpool.tile(shpf, F32, tag="d_y", name="d_y")
                nc.vector.tensor_add(out=t3(d_x[:]), in0=fx, in1=dgx4[:, a:b, :])
                nc.vector.tensor_add(out=t3(d_y[:]), in0=fy, in1=dgy4[:, a:b, :])

                def floor4(src_, tag):
                    # int cast may round on HW; is_gt correction makes floor
                    ii = wpool.tile(shpf, I32, tag=f"{tag}i", name=f"{tag}i")
                    rf = wpool.tile(shpf, F32, tag=f"{tag}r", name=f"{tag}r")
                    gt = wpool.tile(shpf, F32, tag=f"{tag}g", name=f"{tag}g")
                    x0 = wpool.tile(shpf, F32, tag=f"{tag}0", name=f"{tag}0")
                    nc.vector.tensor_copy(out=ii[:], in_=src_[:])
                    nc.vector.tensor_copy(out=rf[:], in_=ii[:])
                    nc.vector.tensor_tensor(out=gt[:], in0=rf[:], in1=src_[:],
                                            op=OP.is_gt)
                    nc.vector.tensor_sub(out=x0[:], in0=rf[:], in1=gt[:])
                    # clamp offset-floor to taps [-2, 1]: extrapolate rare
                    # out-of-band corners instead of dropping them
                    nc.vector.tensor_scalar(out=x0[:], in0=x0[:], scalar1=2.0,
                                            scalar2=5.0, op0=OP.max, op1=OP.min)
                    return x0

                x0f = floor4(d_x, "fx")
                y0f = floor4(d_y, "fy")

                wx1 = wpool.tile(shpf, F16, tag="wx1", name="wx1")
                wy1 = wpool.tile(shpf, F16, tag="wy1", name="wy1")
                wx0 = wpool.tile(shpf, F16, tag="wx0", name="wx0")
                wy0 = wpool.tile(shpf, F16, tag="wy0", name="wy0")
                nc.vector.tensor_sub(out=wx1[:], in0=d_x[:], in1=x0f[:])
                nc.vector.tensor_sub(out=wy1[:], in0=d_y[:], in1=y0f[:])
                nc.vector.tensor_scalar(out=wx0[:], in0=wx1[:], scalar1=-1.0,
                                        scalar2=1.0, op0=OP.mult, op1=OP.add)
                nc.vector.tensor_scalar(out=wy0[:], in0=wy1[:], scalar1=-1.0,
                                        scalar2=1.0, op0=OP.mult, op1=OP.add)

                # softmax numerators; logit bias enters as the e^bwt factor
                e4r = wpool.tile(shpf, F16, tag="e4r", name="e4r")
                nc.scalar.activation(t3(e4r[:]), lg, AF.Exp)
                e4 = wpool.tile(shpf, F16, tag="e4", name="e4")
                nc.vector.tensor_mul(out=t3(e4[:]), in0=t3(e4r[:]),
                                     in1=ek4[:, a:b, :])
                ssum = wpool.tile([128, gsz], F32, tag="ssum", name="ssum")
                nc.vector.tensor_reduce(out=ssum[:], in_=t3(e4[:]),
                                        axis=mybir.AxisListType.X, op=OP.add)
                recb = wpool.tile(shpf, F16, tag="recb", name="recb")
                for k in range(4):
                    nc.vector.reciprocal(t3(recb[:])[:, :, k], ssum[:])
                e4n = wpool.tile(shpf, F16, tag="e4n", name="e4n")
                nc.vector.tensor_mul(out=e4n[:], in0=e4[:], in1=recb[:])
                wy1e = wpool.tile(shpf, F16, tag="wy1e", name="wy1e")
                wy0e = wpool.tile(shpf, F16, tag="wy0e", name="wy0e")
                nc.vector.tensor_mul(out=wy1e[:], in0=wy1[:], in1=e4n[:])
                nc.vector.tensor_mul(out=wy0e[:], in0=wy0[:], in1=e4n[:])

                def taps(x0, w0t, w1t, tag):
                    # tp[v] = (x0==v+4)*w0 + (x0==v+3)*w1 for v in -2..2
                    tp = {}
                    tmp = wpool.tile(shpf, F16, tag=f"{tag}tmp", name=f"{tag}tmp")
                    for v in DXS:
                        h = wpool.tile(shpf, F16, tag=f"{tag}{v}", name=f"{tag}{v}")
                        if v == -2:
                            nc.vector.scalar_tensor_tensor(
                                out=h[:], in0=x0[:], scalar=2.0, in1=w0t[:],
                                op0=OP.is_equal, op1=OP.mult)
                        elif v == 2:
                            nc.vector.scalar_tensor_tensor(
                                out=h[:], in0=x0[:], scalar=5.0, in1=w1t[:],
                                op0=OP.is_equal, op1=OP.mult)
                        else:
                            nc.vector.scalar_tensor_tensor(
                                out=h[:], in0=x0[:], scalar=float(v + 4),
                                in1=w0t[:], op0=OP.is_equal, op1=OP.mult)
                            nc.vector.scalar_tensor_tensor(
                                out=tmp[:], in0=x0[:], scalar=float(v + 3),
                                in1=w1t[:], op0=OP.is_equal, op1=OP.mult)
                            nc.vector.tensor_add(out=h[:], in0=h[:], in1=tmp[:])
                        tp[v] = h
                    return tp

                hx = taps(x0f, wx0, wx1, "hx")
                vy = taps(y0f, wy0e, wy1e, "vy")

                # quad-batched products: 4 s-planes share one X-reduce
                prodq = wpool.tile([128, gsz, 4, 4], F16, tag="prodq",
                                   name="prodq")
                for s0 in range(0, NS, 4):
                    s1 = min(NS, s0 + 4)
                    for s in range(s0, s1):
                        dyv, dxv = SHIFTS[s]
                        nc.vector.tensor_mul(out=prodq[:, :, s - s0, :],
                                             in0=t3(vy[dyv][:]),
                                             in1=t3(hx[dxv][:]))
                    nc.vector.tensor_reduce(
                        out=planes_ng[:, s0:s1, GUARD + a:GUARD + b]
                            .transpose([0, 2, 1]),
                        in_=prodq[:, :, 0:s1 - s0, :],
                        axis=mybir.AxisListType.X, op=OP.add)

            def stage_C(ci):
                """Partition-rotation n->m via TensorE for batch ci."""
                t0c, t1c = CBATCH[ci]
                tb = t1c - t0c
                for si, (s0, s1) in enumerate(SCHUNK):
                    ps = shpsum.tile([128, 13, tb], F32, tag="sh",
                                     name="sh", bufs=1)
                    for s in range(s0, s1):
                        dyv, dxv = SHIFTS[s]
                        delta = dyv * W + dxv
                        b = delta % 128
                        a = (delta - b) // 128
                        # piece 1: rows q>=b <- planes_n[q-b, t-a]; rest 0
                        nc.tensor.matmul(
                            ps[:, s - s0, :],
                            dop[:, 128 - b:256 - b],
                            planes_ng[:, s, GUARD + t0c - a:GUARD + t1c - a],
                            start=True, stop=(b == 0))
                        # piece 2: rows q<b += planes_n[128-b+q, t-a-1]
                        if b > 0:
                            nc.tensor.matmul(
                                ps[:, s - s0, :],
                                dop[:, 256 - b:384 - b],
                                planes_ng[:, s,
                                          GUARD + t0c - a - 1:GUARD + t1c - a - 1],
                                start=False, stop=True)
                    nc.scalar.activation(planes_m[:, s0:s1, t0c:t1c],
                                         ps[:, 0:s1 - s0, :], AF.Copy)

            a_pairs = [None] * NPAIR

            def repack(p0, p1):
                """mp[:, t, s] <- planes_m[:, s, t] for pairs [p0, p1)."""
                nc.gpsimd.tensor_copy(
                    out=mp[:, 2 * p0:2 * p1, 0:NS],
                    in_=planes_m[:, 0:NS, 2 * p0:2 * p1].transpose([0, 2, 1]),
                )

            def scatter(p):
                at = apool.tile([128, 2 * AW], F16, tag="a")
                nc.gpsimd.local_scatter(at[:], mp[:, 2 * p:2 * p + 2, :],
                                        idxp[:, p, :], channels=128,
                                        num_elems=2 * AW, num_idxs=2 * NSP)
                a_pairs[p] = at

            def stage_E(p0, p1):
                for c0 in range(p0, p1, 4):
                    repack(c0, min(p1, c0 + 4))
                    for p in range(c0, min(p1, c0 + 4)):
                        scatter(p)

            def a_cols(r, j0, j1):
                # even r at buffer cols [0, AW) (j-aligned, [0, APAD) zero);
                # odd r data at [AW, 2*AW-APAD) holding j in [APAD, AW)
                at = a_pairs[r // 2]
                off = (r % 2) * (AW - APAD)
                return at[:, off + j0:off + j1]

            def stage_F(b):
                """Main contraction for output block b, o-halves interleaved
                across two PSUM banks to hide accumulation-chain latency."""
                B = 512 * b
                rs = list(range(max(0, 4 * b - 2), min(NT, 4 * b + 6)))
                r_full = 4 * b + 2           # window [B, B+578) covers the block
                prog = [(r_full, B, B + 512)]
                for r in rs:
                    if r == r_full:
                        continue
                    w0 = 128 * r - WOFF
                    n0, n1 = max(B, w0 + APAD), min(B + 512, w0 + AW)
                    if n1 > n0:
                        prog.append((r, n0, n1))
                po = [opsum.tile([128, 512], F32, tag=f"po{oh}", name=f"po{oh}")
                      for oh in range(2)]
                for i, (r, n0, n1) in enumerate(prog):
                    w0 = 128 * r - WOFF
                    for oh in range(2):
                        nc.tensor.matmul(
                            po[oh][:, n0 - B:n1 - B],
                            pfbuf[:, r, ts(oh, 128)],
                            a_cols(r, n0 - w0, n1 - w0),
                            start=(i == 0),
                            stop=(i == len(prog) - 1),
                        )
                for oh in range(2):
                    ob = opool.tile([128, 512], F16, tag="ob", name="ob")
                    nc.scalar.activation(ob[:], po[oh][:], AF.Copy)
                    nc.sync.dma_start(out=out_d[ts(oh, 128), ts(b, 512)],
                                      in_=ob[:])

            # ---------- schedule ----------
            stage_A(0)
            stage_A(1)
            stage_A(2)
            stage_B(0)          # tiles [0, 36): needs pfbuf <= 35 (A0-A2)
            stage_A(3)
            stage_C(0)          # rotation for tiles [0, 34)
            stage_E(0, 17)      # pairs 0-16 (tiles 0-33)
            stage_A(4)
            stage_B(1)          # tiles [36, 72)
            for b in range(0, 3):
                stage_F(b)      # pairs <= 6
            stage_C(1)          # rotation for tiles [34, 72)
            stage_E(17, NPAIR)  # pairs 17-35
            for b in range(3, NBLK):
                stage_F(b)      # F3-F7 pairs <= 16; F8+ from E2
    nc.finalize()
    return nc


_CACHE = {}


def _get_program(inputs):
    key = "prog"
    if key not in _CACHE:
        _CACHE[key] = build_program(
            np.asarray(inputs["Wc"], np.float32),
            np.asarray(inputs["bc"], np.float32),
            np.asarray(inputs["Woff"], np.float32),
            np.asarray(inputs["boff"], np.float32),
            np.asarray(inputs["Wwt"], np.float32),
            np.asarray(inputs["bwt"], np.float32),
        )
    return _CACHE[key]


def kernel(x, Wc, bc, Woff, boff, Wwt, bwt, _trace=False):
    from concourse.bass_utils import run_bass_kernel_spmd

    x = np.asarray(x, np.float32)
    b = x.shape[0]
    assert x.shape == (b, C, H, W) and b == 8

    nc = _get_program(dict(Wc=Wc, bc=bc, Woff=Woff, boff=boff, Wwt=Wwt, bwt=bwt))
    in_maps = [
        {"x": np.ascontiguousarray(x[i].reshape(C, N).astype(np.float16))}
        for i in range(b)
    ]
    res = run_bass_kernel_spmd(nc, in_maps, core_ids=list(range(b)), trace=_trace)
    _CACHE["last_results"] = res
    out = np.stack([res.results[i]["out"].reshape(O, H, W) for i in range(b)])
    return out.astype(np.float32)


# revision 27
# speedup vs baseline: 1.1265x; 1.0462x over previous
"""Trainium2 Bass kernel for nn_DFMAtt: deformable-flow attention.

Per sample (1x1-conv proj, K=4 flow fields, softmax weights, bilinear
grid-sample of proj at flow-displaced positions, weighted sum over K).

Strategy (one batch sample per NeuronCore, 8 cores data-parallel):
  Flows are tiny, so every bilinear corner lies in a fixed 5x5 window
  dy,dx in [-2,2] around its output pixel.  The whole gather-and-blend
  becomes out = proj @ A with A banded (25 diagonals).  Pipeline
  (software-pipelined so all engines overlap):
    - fused [proj | flows | logits] matmul per 128-position tile (f=268),
      bias folded into the PSUM->SBUF copy (DVE tensor_tensor add),
    - fp16 corner-weight planes on DVE (scalar_tensor_tensor fusions),
      softmax normalization folded into e^logits, two half-size batches,
    - partition-shift into source-index space via TensorE rotation
      matmuls against identity slices (PSUM), NOT per-partition DMAs,
    - per-pair banded blocks A [128 x 2*578] via gpsimd.local_scatter;
      border validity is baked into the per-tile scatter indices as -1,
    - main contraction on TensorE fp16, fp16 output.
"""

import os
import sys

sys.path.insert(0, "/opt/trn_rl_repo")

import numpy as np

import concourse.bass as bass
import concourse.mybir as mybir
from concourse import bacc
from concourse.bass import ts
from concourse.tile import TileContext

H = W = 96
C = 256
O = 256
K = 4
N = H * W            # 9216
NT = N // 128        # 72 position tiles
ALPHA = float(W) / float(W - 1)
DYS = list(range(-2, 3))   # -2..2
DXS = list(range(-2, 3))   # -2..2
SHIFTS = [(dy, dx) for dy in DYS for dx in DXS]
NS = len(SHIFTS)     # 25
NSP = 26             # padded (local_scatter needs even num_idxs)
WOFF = 256           # A_r covers n in [r*128 - WOFF, r*128 - WOFF + AW)
AW = 578             # window width; j = q + WOFF - delta_s in [62, 578)
                     # (AW > 516 so r=4b+2 fully covers block b -> single
                     # start=True per PSUM accumulation group)
NBLK = N // 512      # 18 output column blocks
NPAIR = NT // 2      # 36 scatter pairs (2 tiles per local_scatter)
AGRP = [4, 14, 18, 18, 18]     # fused-matmul groups (x-DMA granularity)
BHALF = [(0, 36), (36, 72)]    # plane-pipeline batches
CBATCH = [(0, 34), (34, 72)]   # rotation batches (need planes_n <= t1+2)
SCHUNK = [(0, 13), (13, NS)]   # rotation PSUM s-splits (<=2KB/bank)
APAD = 62                      # zero-pad cols at a-pair front: j>=62 always,
                               # so scatters skip the structurally-zero lead
GUARD = 2                      # zero guard tiles each side of planes_n
FUSED = O + 3 * K    # 268 = proj | fx | fy | logits

F32 = mybir.dt.float32
F16 = mybir.dt.float16
I16 = mybir.dt.int16
I32 = mybir.dt.int32
OP = mybir.AluOpType
AF = mybir.ActivationFunctionType


def _host_consts(Wc, bc, Woff, boff, Wwt, bwt):
    """Host-side constant tensors baked into the NEFF."""
    # fused weight matrix [256, 268]: [Wc^T | a*Woff_x | a*Woff_y | Wwt^T]
    wf = np.concatenate(
        [
            Wc.T.astype(np.float32),                       # [c, 256]
            (ALPHA * Woff[:, 0, :]).T.astype(np.float32),  # [c, 4] fx_k
            (ALPHA * Woff[:, 1, :]).T.astype(np.float32),  # [c, 4] fy_k
            Wwt.T.astype(np.float32),                      # [c, 4]
        ],
        axis=1,
    ).astype(np.float16)
    pbias = np.concatenate([bc.astype(np.float32),
                            np.zeros(3 * K, np.float32)]).astype(np.float16)
    biasbc = np.broadcast_to(pbias[None, :], (128, FUSED)).copy()
    biasrow = pbias[None, :].copy()                    # [1, 268]
    ones = np.ones((1, 128), dtype=np.float16)

    # position fields: n = t*128 + p  ->  F[p, t]; d = ix - gx = fields_x + (a-1)gx
    n_grid = np.arange(N, dtype=np.int64).reshape(NT, 128).T   # [128, 72]
    gx = (n_grid % W).astype(np.float64)
    gy = (n_grid // W).astype(np.float64)

    def rep4(f):  # [128, 72] -> [128, 72, 4]
        return np.repeat(f[:, :, None].astype(np.float32), 4, axis=2)

    # +4.0 biases d into (1, 7) so float->int truncation == floor; the
    # flow-field biases (a*boff - 0.5) fold in per-k.  fp32: fp16 ulp at
    # ~5 is 4e-3, too coarse for bilinear weights.
    dgx4 = rep4((ALPHA - 1.0) * gx + 4.0) + (ALPHA * boff[:, 0] - 0.5
                                             ).astype(np.float32)[None, None, :]
    dgy4 = rep4((ALPHA - 1.0) * gy + 4.0) + (ALPHA * boff[:, 1] - 0.5
                                             ).astype(np.float32)[None, None, :]
    # softmax logit bias as a multiplicative e^bwt factor
    ek4 = np.broadcast_to(np.exp(bwt).astype(np.float16)[None, None, :],
                          (128, NT, 4)).copy()

    # rotation operator bank [0_128 | I | 0_128]: column slices give the
    # shifted identities for both rotation pieces (see stage_C)
    dop = np.zeros((128, 384), dtype=np.float16)
    dop[:, 128:256] = np.eye(128, dtype=np.float16)

    # scatter indices per tile pair, with x-wrap / n-range validity as -1.
    # pair p covers r = 2p (cols 0..AW-1) and r = 2p+1 (cols AW..2AW-1).
    deltas = np.array([dy * W + dx for dy, dx in SHIFTS], dtype=np.int64)
    idxp = np.full((128, NPAIR, 2 * NSP), -1, dtype=np.int16)
    for p in range(NPAIR):
        for half in range(2):
            r = 2 * p + half
            for s, (dy, dx) in enumerate(SHIFTS):
                d = deltas[s]
                for q in range(128):
                    n = r * 128 + q - d          # source output position
                    if n < 0 or n >= N:
                        continue                 # never read (col clipped)
                    if not (0 <= (n % W) + dx <= W - 1):
                        continue                 # x-wrap invalid tap
                    j = q + WOFF - d
                    assert APAD <= j < AW
                    idxp[q, p, half * NSP + s] = j - APAD + half * (AW - APAD)
    return wf, biasbc, biasrow, ones, dgx4, dgy4, ek4, dop, idxp


def build_program(Wc, bc, Woff, boff, Wwt, bwt):
    wf_np, biasbc_np, biasrow_np, ones_np, dgx4_np, dgy4_np, ek4_np, dop_np, idxp_np = _host_consts(
        Wc, bc, Woff, boff, Wwt, bwt)

    nc = bacc.Bacc()
    x_in = nc.dram_tensor("x", [C, N], F16, kind="ExternalInput")
    out_d = nc.dram_tensor("out", [O, N], F16, kind="ExternalOutput")

    wf_d = nc.inline_tensor(wf_np, "wf_c")
    biasbc_d = nc.inline_tensor(biasbc_np, "biasbc_c")
    biasrow_d = nc.inline_tensor(biasrow_np, "biasrow_c")
    ones_d = nc.inline_tensor(ones_np, "ones_c")
    ek4_d = nc.inline_tensor(ek4_np, "ek4_c")
    dgx4_d = nc.inline_tensor(dgx4_np, "dgx4_c")
    dgy4_d = nc.inline_tensor(dgy4_np, "dgy4_c")
    dop_d = nc.inline_tensor(dop_np, "dop_c")
    idxp_d = nc.inline_tensor(idxp_np, "idxp_c")

    agst = []
    t0 = 0
    for gsz in AGRP:
        agst.append(t0)
        t0 += gsz

    with TileContext(nc) as tc, nc.allow_low_precision(reason="f16 bilinear weights"):
        with (
            tc.tile_pool(name="consts", bufs=1) as cpool,
            tc.tile_pool(name="big", bufs=1) as big,
            tc.tile_pool(name="apool", bufs=14) as apool,
            tc.tile_pool(name="work", bufs=2) as wpool,
            tc.tile_pool(name="opool", bufs=4) as opool,
            tc.tile_pool(name="ppsum", bufs=3, space="PSUM") as ppsum,
            tc.tile_pool(name="opsum", bufs=2, space="PSUM") as opsum,
            tc.tile_pool(name="shpsum", bufs=2, space="PSUM") as shpsum,
        ):
            # ---- constants + input, interleaved across both HW DGE queues
            # so stage_A can start ~3us in: weights first, then x chunks
            # (halves split sync/scalar), bulky late-use consts last.
            wf = cpool.tile([128, 2, FUSED], F16, tag="wf")
            nc.sync.dma_start(out=wf[:, 0], in_=wf_d[0:128, :])
            nc.sync.dma_start(out=wf[:, 1], in_=wf_d[128:256, :])
            biasbc = cpool.tile([128, FUSED], F16, tag="biasbc")
            nc.scalar.dma_start(out=biasbc[:], in_=biasbc_d[:])
            biasrow = cpool.tile([1, FUSED], F16, tag="biasrow")
            nc.scalar.dma_start(out=biasrow[:], in_=biasrow_d[:])
            ones_sb = cpool.tile([1, 128], F16, tag="ones_sb")
            nc.scalar.dma_start(out=ones_sb[:], in_=ones_d[:])
            ek4 = cpool.tile([128, NT, 4], F16, tag="ek4")
            nc.scalar.dma_start(out=ek4[:], in_=ek4_d[:])
            xg = []
            for g, gsz in enumerate(AGRP):
                xt = big.tile([128, 2, gsz * 128], F16, tag=f"xg{g}", name=f"xg{g}")
                xg.append(xt)
            dgx4 = cpool.tile([128, NT, 4], F32, tag="dgx4")
            dgy4 = cpool.tile([128, NT, 4], F32, tag="dgy4")
            dop = cpool.tile([128, 384], F16, tag="dop")
            idxp = cpool.tile([128, NPAIR, 2 * NSP], I16, tag="idxp")

            def xdma(g):
                c0 = agst[g] * 128
                c1 = c0 + AGRP[g] * 128
                nc.sync.dma_start(out=xg[g][:, 0], in_=x_in[0:128, c0:c1])
                nc.scalar.dma_start(out=xg[g][:, 1], in_=x_in[128:256, c0:c1])

            xdma(0)
            xdma(1)
            nc.sync.dma_start(out=dgx4[:], in_=dgx4_d[:])
            nc.scalar.dma_start(out=dgy4[:], in_=dgy4_d[:])
            xdma(2)
            xdma(3)
            nc.sync.dma_start(out=dop[:], in_=dop_d[:])
            nc.scalar.dma_start(out=idxp[:], in_=idxp_d[:])
            for g in range(4, len(AGRP)):
                xdma(g)

            pfbuf = big.tile([128, NT, FUSED], F16, tag="pfbuf")
            # planes_n with GUARD zero tiles each side (rotation halo)
            planes_ng = big.tile([128, NS, NT + 2 * GUARD], F16, tag="planes_ng")
            planes_m = big.tile([128, NS, NT], F16, tag="planes_m")
            mp = big.tile([128, NT, NSP], F16, tag="mp")
            nc.vector.memset(planes_ng[:, :, 0:GUARD], 0.0)
            nc.vector.memset(planes_ng[:, :, GUARD + NT:], 0.0)
            nc.vector.memset(mp[:, :, NS:], 0.0)

            # ---------- pipeline stages ----------
            def stage_A(g):
                """Fused [proj|fields] matmuls for group g -> pfbuf (fp16)."""
                for i in range(AGRP[g]):
                    t = agst[g] + i
                    pp = ppsum.tile([128, FUSED], F32, tag="pp")
                    nc.tensor.matmul(pp[:], xg[g][:, 0, ts(i, 128)], wf[:, 0, :],
                                     start=True, stop=False)
                    nc.tensor.matmul(pp[:], xg[g][:, 1, ts(i, 128)], wf[:, 1, :],
                                     start=False, stop=True)
                    # proj bias folded into the DVE copy
                    nc.vector.tensor_add(out=pfbuf[:, t, :], in0=pp[:],
                                         in1=biasbc[:])

            def stage_B(h):
                """Corner-weight planes for tile batch h -> planes_ng.

                Work tiles are flat [128, gsz*4]: dense elementwise ops use
                2D APs (cheaper DVE issue); only pfbuf reads, the k-reduce
                and quad products need 3D views.
                """
                a, b = BHALF[h]
                gsz = b - a
                shpf = [128, gsz * 4]

                def t3(t):  # [128, gsz*4] -> [128, gsz, 4] view
                    return t.rearrange("p (t k) -> p t k", k=4)

                fx = pfbuf[:, a:b, O:O + 4]
                fy = pfbuf[:, a:b, O + 4:O + 8]
                lg = pfbuf[:, a:b, O + 8:O + 12]

                d_x = wpool.tile(shpf, F32, tag="d_x", name="d_x")
                d_y = wpool.tile(shpf, F32, tag="d_y", name="d_y")
                nc.vector.tensor_add(out=t3(d_x[:]), in0=fx, in1=dgx4[:, a:b, :])
                nc.vector.tensor_add(out=t3(d_y[:]), in0=fy, in1=dgy4[:, a:b, :])

                def floor4(src_, tag):
                    # int cast may round on HW; is_gt correction makes floor
                    ii = wpool.tile(shpf, I32, tag=f"{tag}i", name=f"{tag}i")
                    rf = wpool.tile(shpf, F32, tag=f"{tag}r", name=f"{tag}r")
                    gt = wpool.tile(shpf, F32, tag=f"{tag}g", name=f"{tag}g")
                    x0 = wpool.tile(shpf, F32, tag=f"{tag}0", name=f"{tag}0")
                    nc.vector.tensor_copy(out=ii[:], in_=src_[:])
                    nc.vector.tensor_copy(out=rf[:], in_=ii[:])
                    nc.vector.tensor_tensor(out=gt[:], in0=rf[:], in1=src_[:],
                                            op=OP.is_gt)
                    nc.vector.tensor_sub(out=x0[:], in0=rf[:], in1=gt[:])
                    # clamp offset-floor to taps [-2, 1]: extrapolate rare
                    # out-of-band corners instead of dropping them
                    nc.vector.tensor_scalar(out=x0[:], in0=x0[:], scalar1=2.0,
                                            scalar2=5.0, op0=OP.max, op1=OP.min)
                    return x0

                x0f = floor4(d_x, "fx")
                y0f = floor4(d_y, "fy")

                wx1 = wpool.tile(shpf, F16, tag="wx1", name="wx1")
                wy1 = wpool.tile(shpf, F16, tag="wy1", name="wy1")
                wx0 = wpool.tile(shpf, F16, tag="wx0", name="wx0")
                wy0 = wpool.tile(shpf, F16, tag="wy0", name="wy0")
                nc.vector.tensor_sub(out=wx1[:], in0=d_x[:], in1=x0f[:])
                nc.vector.tensor_sub(out=wy1[:], in0=d_y[:], in1=y0f[:])
                nc.vector.tensor_scalar(out=wx0[:], in0=wx1[:], scalar1=-1.0,
                                        scalar2=1.0, op0=OP.mult, op1=OP.add)
                nc.vector.tensor_scalar(out=wy0[:], in0=wy1[:], scalar1=-1.0,
                                        scalar2=1.0, op0=OP.mult, op1=OP.add)

                # softmax numerators; logit bias enters as the e^bwt factor
                e4r = wpool.tile(shpf, F16, tag="e4r", name="e4r")
                nc.scalar.activation(t3(e4r[:]), lg, AF.Exp)
                e4 = wpool.tile(shpf, F16, tag="e4", name="e4")
                nc.vector.tensor_mul(out=t3(e4[:]), in0=t3(e4r[:]),
                                     in1=ek4[:, a:b, :])
                ssum = wpool.tile([128, gsz], F32, tag="ssum", name="ssum")
                nc.vector.tensor_reduce(out=ssum[:], in_=t3(e4[:]),
                                        axis=mybir.AxisListType.X, op=OP.add)
                recb = wpool.tile(shpf, F16, tag="recb", name="recb")
                for k in range(4):
                    nc.vector.reciprocal(t3(recb[:])[:, :, k], ssum[:])
                e4n = wpool.tile(shpf, F16, tag="e4n", name="e4n")
                nc.vector.tensor_mul(out=e4n[:], in0=e4[:], in1=recb[:])
                wy1e = wpool.tile(shpf, F16, tag="wy1e", name="wy1e")
                wy0e = wpool.tile(shpf, F16, tag="wy0e", name="wy0e")
                nc.vector.tensor_mul(out=wy1e[:], in0=wy1[:], in1=e4n[:])
                nc.vector.tensor_mul(out=wy0e[:], in0=wy0[:], in1=e4n[:])

                def taps(x0, w0t, w1t, tag):
                    # tp[v] = (x0==v+4)*w0 + (x0==v+3)*w1 for v in -2..2
                    tp = {}
                    tmp = wpool.tile(shpf, F16, tag=f"{tag}tmp", name=f"{tag}tmp")
                    for v in DXS:
                        h = wpool.tile(shpf, F16, tag=f"{tag}{v}", name=f"{tag}{v}")
                        if v == -2:
                            nc.vector.scalar_tensor_tensor(
                                out=h[:], in0=x0[:], scalar=2.0, in1=w0t[:],
                                op0=OP.is_equal, op1=OP.mult)
                        elif v == 2:
                            nc.vector.scalar_tensor_tensor(
                                out=h[:], in0=x0[:], scalar=5.0, in1=w1t[:],
                                op0=OP.is_equal, op1=OP.mult)
                        else:
                            nc.vector.scalar_tensor_tensor(
                                out=h[:], in0=x0[:], scalar=float(v + 4),
                                in1=w0t[:], op0=OP.is_equal, op1=OP.mult)
                            nc.vector.scalar_tensor_tensor(
                                out=tmp[:], in0=x0[:], scalar=float(v + 3),
                                in1=w1t[:], op0=OP.is_equal, op1=OP.mult)
                            nc.vector.tensor_add(out=h[:], in0=h[:], in1=tmp[:])
                        tp[v] = h
                    return tp

                hx = taps(x0f, wx0, wx1, "hx")
                vy = taps(y0f, wy0e, wy1e, "vy")

                # quad-batched products: 4 s-planes share one X-reduce
                prodq = wpool.tile([128, gsz, 4, 4], F16, tag="prodq",
                                   name="prodq")
                for s0 in range(0, NS, 4):
                    s1 = min(NS, s0 + 4)
                    for s in range(s0, s1):
                        dyv, dxv = SHIFTS[s]
                        nc.vector.tensor_mul(out=prodq[:, :, s - s0, :],
                                             in0=t3(vy[dyv][:]),
                                             in1=t3(hx[dxv][:]))
                    nc.vector.tensor_reduce(
                        out=planes_ng[:, s0:s1, GUARD + a:GUARD + b]
                            .transpose([0, 2, 1]),
                        in_=prodq[:, :, 0:s1 - s0, :],
                        axis=mybir.AxisListType.X, op=OP.add)

            def stage_C(ci):
                """Partition-rotation n->m via TensorE for batch ci."""
                t0c, t1c = CBATCH[ci]
                tb = t1c - t0c
                for si, (s0, s1) in enumerate(SCHUNK):
                    ps = shpsum.tile([128, 13, tb], F32, tag="sh",
                                     name="sh", bufs=1)
                    for s in range(s0, s1):
                        dyv, dxv = SHIFTS[s]
                        delta = dyv * W + dxv
                        b = delta % 128
                        a = (delta - b) // 128
                        # piece 1: rows q>=b <- planes_n[q-b, t-a]; rest 0
                        nc.tensor.matmul(
                            ps[:, s - s0, :],
                            dop[:, 128 - b:256 - b],
                            planes_ng[:, s, GUARD + t0c - a:GUARD + t1c - a],
                            start=True, stop=(b == 0))
                        # piece 2: rows q<b += planes_n[128-b+q, t-a-1]
                        if b > 0:
                            nc.tensor.matmul(
                                ps[:, s - s0, :],
                                dop[:, 256 - b:384 - b],
                                planes_ng[:, s,
                                          GUARD + t0c - a - 1:GUARD + t1c - a - 1],
                                start=False, stop=True)
                    nc.scalar.activation(planes_m[:, s0:s1, t0c:t1c],
                                         ps[:, 0:s1 - s0, :], AF.Copy)

            a_pairs = [None] * NPAIR

            def repack(p0, p1):
                """mp[:, t, s] <- planes_m[:, s, t] for pairs [p0, p1)."""
                nc.gpsimd.tensor_copy(
                    out=mp[:, 2 * p0:2 * p1, 0:NS],
                    in_=planes_m[:, 0:NS, 2 * p0:2 * p1].transpose([0, 2, 1]),
                )

            def scatter(p):
                at = apool.tile([128, 2 * AW - APAD], F16, tag="a")
                nc.vector.memset(at[:, 0:APAD], 0.0)
                nc.gpsimd.local_scatter(at[:, APAD:], mp[:, 2 * p:2 * p + 2, :],
                                        idxp[:, p, :], channels=128,
                                        num_elems=2 * (AW - APAD),
                                        num_idxs=2 * NSP)
                a_pairs[p] = at

            def stage_E(p0, p1):
                for c0 in range(p0, p1, 4):
                    repack(c0, min(p1, c0 + 4))
                    for p in range(c0, min(p1, c0 + 4)):
                        scatter(p)

            def a_cols(r, j0, j1):
                # even r at buffer cols [0, AW) (j-aligned, [0, APAD) zero);
                # odd r data at [AW, 2*AW-APAD) holding j in [APAD, AW)
                at = a_pairs[r // 2]
                off = (r % 2) * (AW - APAD)
                return at[:, off + j0:off + j1]

            def stage_F(b):
                """Main contraction for output block b, o-halves interleaved
                across two PSUM banks to hide accumulation-chain latency."""
                B = 512 * b
                rs = list(range(max(0, 4 * b - 2), min(NT, 4 * b + 6)))
                r_full = 4 * b + 2           # window [B, B+578) covers the block
                prog = [(r_full, B, B + 512)]
                for r in rs:
                    if r == r_full:
                        continue
                    w0 = 128 * r - WOFF
                    n0, n1 = max(B, w0 + APAD), min(B + 512, w0 + AW)
                    if n1 > n0:
                        prog.append((r, n0, n1))
                po = [opsum.tile([128, 512], F32, tag=f"po{oh}", name=f"po{oh}")
                      for oh in range(2)]
                for i, (r, n0, n1) in enumerate(prog):
                    w0 = 128 * r - WOFF
                    for oh in range(2):
                        nc.tensor.matmul(
                            po[oh][:, n0 - B:n1 - B],
                            pfbuf[:, r, ts(oh, 128)],
                            a_cols(r, n0 - w0, n1 - w0),
                            start=(i == 0),
                            stop=(i == len(prog) - 1),
                        )
                for oh in range(2):
                    ob = opool.tile([128, 512], F16, tag="ob", name="ob")
                    nc.scalar.activation(ob[:], po[oh][:], AF.Copy)
                    eng = nc.sync if oh == 0 else nc.scalar
                    eng.dma_start(out=out_d[ts(oh, 128), ts(b, 512)], in_=ob[:])

            # ---------- schedule ----------
            stage_A(0)
            stage_A(1)
            stage_A(2)
            stage_B(0)          # tiles [0, 36): needs pfbuf <= 35 (A0-A2)
            stage_A(3)
            stage_C(0)          # rotation for tiles [0, 34)
            stage_E(0, 17)      # pairs 0-16 (tiles 0-33)
            stage_A(4)
            stage_B(1)          # tiles [36, 72)
            for b in range(0, 3):
                stage_F(b)      # pairs <= 6
            stage_C(1)          # rotation for tiles [34, 72)
            stage_E(17, NPAIR)  # pairs 17-35
            for b in range(3, NBLK):
                stage_F(b)      # F3-F7 pairs <= 16; F8+ from E2
    nc.finalize()
    return nc


_CACHE = {}


def _get_program(inputs):
    key = "prog"
    if key not in _CACHE:
        _CACHE[key] = build_program(
            np.asarray(inputs["Wc"], np.float32),
            np.asarray(inputs["bc"], np.float32),
            np.asarray(inputs["Woff"], np.float32),
            np.asarray(inputs["boff"], np.float32),
            np.asarray(inputs["Wwt"], np.float32),
            np.asarray(inputs["bwt"], np.float32),
        )
    return _CACHE[key]


def kernel(x, Wc, bc, Woff, boff, Wwt, bwt, _trace=False):
    from concourse.bass_utils import run_bass_kernel_spmd

    x = np.asarray(x, np.float32)
    b = x.shape[0]
    assert x.shape == (b, C, H, W) and b == 8

    nc = _get_program(dict(Wc=Wc, bc=bc, Woff=Woff, boff=boff, Wwt=Wwt, bwt=bwt))
    in_maps = [
        {"x": np.ascontiguousarray(x[i].reshape(C, N).astype(np.float16))}
        for i in range(b)
    ]
    res = run_bass_kernel_spmd(nc, in_maps, core_ids=list(range(b)), trace=_trace)
    _CACHE["last_results"] = res
    out = np.stack([res.results[i]["out"].reshape(O, H, W) for i in range(b)])
    return out.astype(np.float32)


# revision 29
# speedup vs baseline: 1.1297x; 1.0028x over previous
"""Trainium2 Bass kernel for nn_DFMAtt: deformable-flow attention.

Per sample (1x1-conv proj, K=4 flow fields, softmax weights, bilinear
grid-sample of proj at flow-displaced positions, weighted sum over K).

Strategy (one batch sample per NeuronCore, 8 cores data-parallel):
  Flows are tiny, so every bilinear corner lies in a fixed 5x5 window
  dy,dx in [-2,2] around its output pixel.  The whole gather-and-blend
  becomes out = proj @ A with A banded (25 diagonals).  Pipeline
  (software-pipelined so all engines overlap):
    - fused [proj | flows | logits] matmul per 128-position tile (f=268),
      bias folded into the PSUM->SBUF copy (DVE tensor_tensor add),
    - fp16 corner-weight planes on DVE (scalar_tensor_tensor fusions),
      softmax normalization folded into e^logits, two half-size batches,
    - partition-shift into source-index space via TensorE rotation
      matmuls against identity slices (PSUM), NOT per-partition DMAs,
    - per-pair banded blocks A [128 x 2*578] via gpsimd.local_scatter;
      border validity is baked into the per-tile scatter indices as -1,
    - main contraction on TensorE fp16, fp16 output.
"""

import os
import sys

sys.path.insert(0, "/opt/trn_rl_repo")

import numpy as np

import concourse.bass as bass
import concourse.mybir as mybir
from concourse import bacc
from concourse.bass import ts
from concourse.tile import TileContext

H = W = 96
C = 256
O = 256
K = 4
N = H * W            # 9216
NT = N // 128        # 72 position tiles
ALPHA = float(W) / float(W - 1)
DYS = list(range(-2, 3))   # -2..2
DXS = list(range(-2, 3))   # -2..2
SHIFTS = [(dy, dx) for dy in DYS for dx in DXS]
NS = len(SHIFTS)     # 25
NSP = 26             # padded (local_scatter needs even num_idxs)
WOFF = 256           # A_r covers n in [r*128 - WOFF, r*128 - WOFF + AW)
AW = 578             # window width; j = q + WOFF - delta_s in [62, 578)
                     # (AW > 516 so r=4b+2 fully covers block b -> single
                     # start=True per PSUM accumulation group)
NBLK = N // 512      # 18 output column blocks
NPAIR = NT // 2      # 36 scatter pairs (2 tiles per local_scatter)
AGRP = [4, 14, 18, 18, 18]     # fused-matmul groups (x-DMA granularity)
BHALF = [(0, 36), (36, 72)]    # plane-pipeline batches
CBATCH = [(0, 34), (34, 72)]   # rotation batches (need planes_n <= t1+2)
SCHUNK = [(0, 13), (13, NS)]   # rotation PSUM s-splits (<=2KB/bank)
APAD = 62                      # zero-pad cols at a-pair front: j>=62 always,
                               # so scatters skip the structurally-zero lead
GUARD = 2                      # zero guard tiles each side of planes_n
FUSED = O + 3 * K    # 268 = proj | fx | fy | logits

F32 = mybir.dt.float32
F16 = mybir.dt.float16
I16 = mybir.dt.int16
I32 = mybir.dt.int32
OP = mybir.AluOpType
AF = mybir.ActivationFunctionType


def _host_consts(Wc, bc, Woff, boff, Wwt, bwt):
    """Host-side constant tensors baked into the NEFF."""
    # fused weight matrix [256, 268]: [Wc^T | a*Woff_x | a*Woff_y | Wwt^T]
    wf = np.concatenate(
        [
            Wc.T.astype(np.float32),                       # [c, 256]
            (ALPHA * Woff[:, 0, :]).T.astype(np.float32),  # [c, 4] fx_k
            (ALPHA * Woff[:, 1, :]).T.astype(np.float32),  # [c, 4] fy_k
            Wwt.T.astype(np.float32),                      # [c, 4]
        ],
        axis=1,
    ).astype(np.float16)
    pbias = np.concatenate([bc.astype(np.float32),
                            np.zeros(3 * K, np.float32)]).astype(np.float16)
    biasbc = np.broadcast_to(pbias[None, :], (128, FUSED)).copy()
    biasrow = pbias[None, :].copy()                    # [1, 268]
    ones = np.ones((1, 128), dtype=np.float16)

    # position fields: n = t*128 + p  ->  F[p, t]; d = ix - gx = fields_x + (a-1)gx
    n_grid = np.arange(N, dtype=np.int64).reshape(NT, 128).T   # [128, 72]
    gx = (n_grid % W).astype(np.float64)
    gy = (n_grid // W).astype(np.float64)

    def rep4(f):  # [128, 72] -> [128, 72, 4]
        return np.repeat(f[:, :, None].astype(np.float32), 4, axis=2)

    # +4.0 biases d into (1, 7) so float->int truncation == floor; the
    # flow-field biases (a*boff - 0.5) fold in per-k.  fp32: fp16 ulp at
    # ~5 is 4e-3, too coarse for bilinear weights.
    dgx4 = rep4((ALPHA - 1.0) * gx + 4.0) + (ALPHA * boff[:, 0] - 0.5
                                             ).astype(np.float32)[None, None, :]
    dgy4 = rep4((ALPHA - 1.0) * gy + 4.0) + (ALPHA * boff[:, 1] - 0.5
                                             ).astype(np.float32)[None, None, :]
    # softmax logit bias as a multiplicative e^bwt factor
    ek4 = np.broadcast_to(np.exp(bwt).astype(np.float16)[None, None, :],
                          (128, NT, 4)).copy()

    # rotation operator bank [0_128 | I | 0_128]: column slices give the
    # shifted identities for both rotation pieces (see stage_C)
    dop = np.zeros((128, 384), dtype=np.float16)
    dop[:, 128:256] = np.eye(128, dtype=np.float16)

    # scatter indices per tile pair, with x-wrap / n-range validity as -1.
    # pair p covers r = 2p (cols 0..AW-1) and r = 2p+1 (cols AW..2AW-1).
    deltas = np.array([dy * W + dx for dy, dx in SHIFTS], dtype=np.int64)
    idxp = np.full((128, NPAIR, 2 * NSP), -1, dtype=np.int16)
    for p in range(NPAIR):
        for half in range(2):
            r = 2 * p + half
            for s, (dy, dx) in enumerate(SHIFTS):
                d = deltas[s]
                for q in range(128):
                    n = r * 128 + q - d          # source output position
                    if n < 0 or n >= N:
                        continue                 # never read (col clipped)
                    if not (0 <= (n % W) + dx <= W - 1):
                        continue                 # x-wrap invalid tap
                    j = q + WOFF - d
                    assert APAD <= j < AW
                    idxp[q, p, half * NSP + s] = j - APAD + half * (AW - APAD)
    return wf, biasbc, biasrow, ones, dgx4, dgy4, ek4, dop, idxp


def build_program(Wc, bc, Woff, boff, Wwt, bwt):
    wf_np, biasbc_np, biasrow_np, ones_np, dgx4_np, dgy4_np, ek4_np, dop_np, idxp_np = _host_consts(
        Wc, bc, Woff, boff, Wwt, bwt)

    nc = bacc.Bacc()
    x_in = nc.dram_tensor("x", [C, N], F16, kind="ExternalInput")
    out_d = nc.dram_tensor("out", [O, N], F16, kind="ExternalOutput")

    wf_d = nc.inline_tensor(wf_np, "wf_c")
    biasbc_d = nc.inline_tensor(biasbc_np, "biasbc_c")
    biasrow_d = nc.inline_tensor(biasrow_np, "biasrow_c")
    ones_d = nc.inline_tensor(ones_np, "ones_c")
    ek4_d = nc.inline_tensor(ek4_np, "ek4_c")
    dgx4_d = nc.inline_tensor(dgx4_np, "dgx4_c")
    dgy4_d = nc.inline_tensor(dgy4_np, "dgy4_c")
    dop_d = nc.inline_tensor(dop_np, "dop_c")
    idxp_d = nc.inline_tensor(idxp_np, "idxp_c")

    agst = []
    t0 = 0
    for gsz in AGRP:
        agst.append(t0)
        t0 += gsz

    with TileContext(nc) as tc, nc.allow_low_precision(reason="f16 bilinear weights"):
        with (
            tc.tile_pool(name="consts", bufs=1) as cpool,
            tc.tile_pool(name="big", bufs=1) as big,
            tc.tile_pool(name="apool", bufs=16) as apool,
            tc.tile_pool(name="work", bufs=2) as wpool,
            tc.tile_pool(name="opool", bufs=4) as opool,
            tc.tile_pool(name="ppsum", bufs=3, space="PSUM") as ppsum,
            tc.tile_pool(name="opsum", bufs=2, space="PSUM") as opsum,
            tc.tile_pool(name="shpsum", bufs=2, space="PSUM") as shpsum,
        ):
            # ---- constants + input, interleaved across both HW DGE queues
            # so stage_A can start ~3us in: weights first, then x chunks
            # (halves split sync/scalar), bulky late-use consts last.
            wf = cpool.tile([128, 2, FUSED], F16, tag="wf")
            nc.sync.dma_start(out=wf[:, 0], in_=wf_d[0:128, :])
            nc.sync.dma_start(out=wf[:, 1], in_=wf_d[128:256, :])
            biasbc = cpool.tile([128, FUSED], F16, tag="biasbc")
            nc.scalar.dma_start(out=biasbc[:], in_=biasbc_d[:])
            biasrow = cpool.tile([1, FUSED], F16, tag="biasrow")
            nc.scalar.dma_start(out=biasrow[:], in_=biasrow_d[:])
            ones_sb = cpool.tile([1, 128], F16, tag="ones_sb")
            nc.scalar.dma_start(out=ones_sb[:], in_=ones_d[:])
            ek4 = cpool.tile([128, NT, 4], F16, tag="ek4")
            nc.scalar.dma_start(out=ek4[:], in_=ek4_d[:])
            xg = []
            for g, gsz in enumerate(AGRP):
                xt = big.tile([128, 2, gsz * 128], F16, tag=f"xg{g}", name=f"xg{g}")
                xg.append(xt)
            dgx4 = cpool.tile([128, NT, 4], F32, tag="dgx4")
            dgy4 = cpool.tile([128, NT, 4], F32, tag="dgy4")
            dop = cpool.tile([128, 384], F16, tag="dop")
            idxp = cpool.tile([128, NPAIR, 2 * NSP], I16, tag="idxp")

            def xdma(g):
                c0 = agst[g] * 128
                c1 = c0 + AGRP[g] * 128
                nc.sync.dma_start(out=xg[g][:, 0], in_=x_in[0:128, c0:c1])
                nc.scalar.dma_start(out=xg[g][:, 1], in_=x_in[128:256, c0:c1])

            xdma(0)
            xdma(1)
            nc.sync.dma_start(out=dgx4[:], in_=dgx4_d[:])
            nc.scalar.dma_start(out=dgy4[:], in_=dgy4_d[:])
            xdma(2)
            xdma(3)
            nc.sync.dma_start(out=dop[:], in_=dop_d[:])
            nc.scalar.dma_start(out=idxp[:], in_=idxp_d[:])
            for g in range(4, len(AGRP)):
                xdma(g)

            pfbuf = big.tile([128, NT, FUSED], F16, tag="pfbuf")
            # planes_n with GUARD zero tiles each side (rotation halo)
            planes_ng = big.tile([128, NS, NT + 2 * GUARD], F16, tag="planes_ng")
            planes_m = big.tile([128, NS, NT], F16, tag="planes_m")
            mp = big.tile([128, NT, NSP], F16, tag="mp")
            nc.vector.memset(planes_ng[:, :, 0:GUARD], 0.0)
            nc.vector.memset(planes_ng[:, :, GUARD + NT:], 0.0)
            nc.vector.memset(mp[:, :, NS:], 0.0)

            # ---------- pipeline stages ----------
            def stage_A(g):
                """Fused [proj|fields] matmuls for group g -> pfbuf (fp16)."""
                for i in range(AGRP[g]):
                    t = agst[g] + i
                    pp = ppsum.tile([128, FUSED], F32, tag="pp")
                    nc.tensor.matmul(pp[:], xg[g][:, 0, ts(i, 128)], wf[:, 0, :],
                                     start=True, stop=False)
                    nc.tensor.matmul(pp[:], xg[g][:, 1, ts(i, 128)], wf[:, 1, :],
                                     start=False, stop=True)
                    # proj bias folded into the DVE copy
                    nc.vector.tensor_add(out=pfbuf[:, t, :], in0=pp[:],
                                         in1=biasbc[:])

            def stage_B(h):
                """Corner-weight planes for tile batch h -> planes_ng.

                Work tiles are flat [128, gsz*4]: dense elementwise ops use
                2D APs (cheaper DVE issue); only pfbuf reads, the k-reduce
                and quad products need 3D views.
                """
                a, b = BHALF[h]
                gsz = b - a
                shpf = [128, gsz * 4]

                def t3(t):  # [128, gsz*4] -> [128, gsz, 4] view
                    return t.rearrange("p (t k) -> p t k", k=4)

                fx = pfbuf[:, a:b, O:O + 4]
                fy = pfbuf[:, a:b, O + 4:O + 8]
                lg = pfbuf[:, a:b, O + 8:O + 12]

                d_x = wpool.tile(shpf, F32, tag="d_x", name="d_x")
                d_y = wpool.tile(shpf, F32, tag="d_y", name="d_y")
                nc.vector.tensor_add(out=t3(d_x[:]), in0=fx, in1=dgx4[:, a:b, :])
                nc.vector.tensor_add(out=t3(d_y[:]), in0=fy, in1=dgy4[:, a:b, :])

                def floor4(src_, tag):
                    # int cast may round on HW; is_gt correction makes floor
                    ii = wpool.tile(shpf, I32, tag=f"{tag}i", name=f"{tag}i")
                    rf = wpool.tile(shpf, F32, tag=f"{tag}r", name=f"{tag}r")
                    gt = wpool.tile(shpf, F32, tag=f"{tag}g", name=f"{tag}g")
                    x0 = wpool.tile(shpf, F32, tag=f"{tag}0", name=f"{tag}0")
                    nc.vector.tensor_copy(out=ii[:], in_=src_[:])
                    nc.vector.tensor_copy(out=rf[:], in_=ii[:])
                    nc.vector.tensor_tensor(out=gt[:], in0=rf[:], in1=src_[:],
                                            op=OP.is_gt)
                    nc.vector.tensor_sub(out=x0[:], in0=rf[:], in1=gt[:])
                    # clamp offset-floor to taps [-2, 1]: extrapolate rare
                    # out-of-band corners instead of dropping them
                    nc.vector.tensor_scalar(out=x0[:], in0=x0[:], scalar1=2.0,
                                            scalar2=5.0, op0=OP.max, op1=OP.min)
                    return x0

                x0f = floor4(d_x, "fx")
                y0f = floor4(d_y, "fy")

                wx1 = wpool.tile(shpf, F16, tag="wx1", name="wx1")
                wy1 = wpool.tile(shpf, F16, tag="wy1", name="wy1")
                wx0 = wpool.tile(shpf, F16, tag="wx0", name="wx0")
                wy0 = wpool.tile(shpf, F16, tag="wy0", name="wy0")
                nc.vector.tensor_sub(out=wx1[:], in0=d_x[:], in1=x0f[:])
                nc.vector.tensor_sub(out=wy1[:], in0=d_y[:], in1=y0f[:])
                nc.vector.tensor_scalar(out=wx0[:], in0=wx1[:], scalar1=-1.0,
                                        scalar2=1.0, op0=OP.mult, op1=OP.add)
                nc.vector.tensor_scalar(out=wy0[:], in0=wy1[:], scalar1=-1.0,
                                        scalar2=1.0, op0=OP.mult, op1=OP.add)

                # softmax numerators; logit bias enters as the e^bwt factor
                e4r = wpool.tile(shpf, F16, tag="e4r", name="e4r")
                nc.scalar.activation(t3(e4r[:]), lg, AF.Exp)
                e4 = wpool.tile(shpf, F16, tag="e4", name="e4")
                nc.vector.tensor_mul(out=t3(e4[:]), in0=t3(e4r[:]),
                                     in1=ek4[:, a:b, :])
                ssum = wpool.tile([128, gsz], F32, tag="ssum", name="ssum")
                nc.vector.tensor_reduce(out=ssum[:], in_=t3(e4[:]),
                                        axis=mybir.AxisListType.X, op=OP.add)
                recb = wpool.tile(shpf, F16, tag="recb", name="recb")
                for k in range(4):
                    nc.vector.reciprocal(t3(recb[:])[:, :, k], ssum[:])
                e4n = wpool.tile(shpf, F16, tag="e4n", name="e4n")
                nc.vector.tensor_mul(out=e4n[:], in0=e4[:], in1=recb[:])
                wy1e = wpool.tile(shpf, F16, tag="wy1e", name="wy1e")
                wy0e = wpool.tile(shpf, F16, tag="wy0e", name="wy0e")
                nc.vector.tensor_mul(out=wy1e[:], in0=wy1[:], in1=e4n[:])
                nc.vector.tensor_mul(out=wy0e[:], in0=wy0[:], in1=e4n[:])

                def taps(x0, w0t, w1t, tag):
                    # tp[v] = (x0==v+4)*w0 + (x0==v+3)*w1 for v in -2..2
                    tp = {}
                    tmp = wpool.tile(shpf, F16, tag=f"{tag}tmp", name=f"{tag}tmp")
                    for v in DXS:
                        h = wpool.tile(shpf, F16, tag=f"{tag}{v}", name=f"{tag}{v}")
                        if v == -2:
                            nc.vector.scalar_tensor_tensor(
                                out=h[:], in0=x0[:], scalar=2.0, in1=w0t[:],
                                op0=OP.is_equal, op1=OP.mult)
                        elif v == 2:
                            nc.vector.scalar_tensor_tensor(
                                out=h[:], in0=x0[:], scalar=5.0, in1=w1t[:],
                                op0=OP.is_equal, op1=OP.mult)
                        else:
                            nc.vector.scalar_tensor_tensor(
                                out=h[:], in0=x0[:], scalar=float(v + 4),
                                in1=w0t[:], op0=OP.is_equal, op1=OP.mult)
                            nc.vector.scalar_tensor_tensor(
                                out=tmp[:], in0=x0[:], scalar=float(v + 3),
                                in1=w1t[:], op0=OP.is_equal, op1=OP.mult)
                            nc.vector.tensor_add(out=h[:], in0=h[:], in1=tmp[:])
                        tp[v] = h
                    return tp

                hx = taps(x0f, wx0, wx1, "hx")
                vy = taps(y0f, wy0e, wy1e, "vy")

                # quad-batched products: 4 s-planes share one X-reduce
                prodq = wpool.tile([128, gsz, 4, 4], F16, tag="prodq",
                                   name="prodq")
                for s0 in range(0, NS, 4):
                    s1 = min(NS, s0 + 4)
                    for s in range(s0, s1):
                        dyv, dxv = SHIFTS[s]
                        nc.vector.tensor_mul(out=prodq[:, :, s - s0, :],
                                             in0=t3(vy[dyv][:]),
                                             in1=t3(hx[dxv][:]))
                    nc.vector.tensor_reduce(
                        out=planes_ng[:, s0:s1, GUARD + a:GUARD + b]
                            .transpose([0, 2, 1]),
                        in_=prodq[:, :, 0:s1 - s0, :],
                        axis=mybir.AxisListType.X, op=OP.add)

            def stage_C(ci):
                """Partition-rotation n->m via TensorE for batch ci."""
                t0c, t1c = CBATCH[ci]
                tb = t1c - t0c
                for si, (s0, s1) in enumerate(SCHUNK):
                    ps = shpsum.tile([128, 13, tb], F32, tag="sh",
                                     name="sh", bufs=1)
                    for s in range(s0, s1):
                        dyv, dxv = SHIFTS[s]
                        delta = dyv * W + dxv
                        b = delta % 128
                        a = (delta - b) // 128
                        # piece 1: rows q>=b <- planes_n[q-b, t-a]; rest 0
                        nc.tensor.matmul(
                            ps[:, s - s0, :],
                            dop[:, 128 - b:256 - b],
                            planes_ng[:, s, GUARD + t0c - a:GUARD + t1c - a],
                            start=True, stop=(b == 0))
                        # piece 2: rows q<b += planes_n[128-b+q, t-a-1]
                        if b > 0:
                            nc.tensor.matmul(
                                ps[:, s - s0, :],
                                dop[:, 256 - b:384 - b],
                                planes_ng[:, s,
                                          GUARD + t0c - a - 1:GUARD + t1c - a - 1],
                                start=False, stop=True)
                    nc.scalar.activation(planes_m[:, s0:s1, t0c:t1c],
                                         ps[:, 0:s1 - s0, :], AF.Copy)

            a_pairs = [None] * NPAIR

            def repack(p0, p1):
                """mp[:, t, s] <- planes_m[:, s, t] for pairs [p0, p1)."""
                nc.gpsimd.tensor_copy(
                    out=mp[:, 2 * p0:2 * p1, 0:NS],
                    in_=planes_m[:, 0:NS, 2 * p0:2 * p1].transpose([0, 2, 1]),
                )

            def scatter(p):
                at = apool.tile([128, 2 * AW - APAD], F16, tag="a")
                nc.gpsimd.memset(at[:, 0:APAD], 0.0)
                nc.gpsimd.local_scatter(at[:, APAD:], mp[:, 2 * p:2 * p + 2, :],
                                        idxp[:, p, :], channels=128,
                                        num_elems=2 * (AW - APAD),
                                        num_idxs=2 * NSP)
                a_pairs[p] = at

            def stage_E(p0, p1):
                for c0 in range(p0, p1, 4):
                    repack(c0, min(p1, c0 + 4))
                    for p in range(c0, min(p1, c0 + 4)):
                        scatter(p)

            def a_cols(r, j0, j1):
                # even r at buffer cols [0, AW) (j-aligned, [0, APAD) zero);
                # odd r data at [AW, 2*AW-APAD) holding j in [APAD, AW)
                at = a_pairs[r // 2]
                off = (r % 2) * (AW - APAD)
                return at[:, off + j0:off + j1]

            def stage_F(b):
                """Main contraction for output block b, o-halves interleaved
                across two PSUM banks to hide accumulation-chain latency."""
                B = 512 * b
                rs = list(range(max(0, 4 * b - 2), min(NT, 4 * b + 6)))
                r_full = 4 * b + 2           # window [B, B+578) covers the block
                prog = [(r_full, B, B + 512)]
                for r in rs:
                    if r == r_full:
                        continue
                    w0 = 128 * r - WOFF
                    n0, n1 = max(B, w0 + APAD), min(B + 512, w0 + AW)
                    if n1 > n0:
                        prog.append((r, n0, n1))
                po = [opsum.tile([128, 512], F32, tag=f"po{oh}", name=f"po{oh}")
                      for oh in range(2)]
                for i, (r, n0, n1) in enumerate(prog):
                    w0 = 128 * r - WOFF
                    for oh in range(2):
                        nc.tensor.matmul(
                            po[oh][:, n0 - B:n1 - B],
                            pfbuf[:, r, ts(oh, 128)],
                            a_cols(r, n0 - w0, n1 - w0),
                            start=(i == 0),
                            stop=(i == len(prog) - 1),
                        )
                for oh in range(2):
                    ob = opool.tile([128, 512], F16, tag="ob", name="ob")
                    nc.scalar.activation(ob[:], po[oh][:], AF.Copy)
                    eng = nc.sync if oh == 0 else nc.scalar
                    eng.dma_start(out=out_d[ts(oh, 128), ts(b, 512)], in_=ob[:])

            # ---------- schedule ----------
            stage_A(0)
            stage_A(1)
            stage_A(2)
            stage_B(0)          # tiles [0, 36): needs pfbuf <= 35 (A0-A2)
            stage_A(3)
            stage_A(4)
            stage_C(0)          # rotation for tiles [0, 34)
            stage_E(0, 17)      # pairs 0-16 (tiles 0-33)
            stage_B(1)          # tiles [36, 72)
            for b in range(0, 8):
                stage_F(b)      # needs pairs <= 2b+2 <= 16
            stage_C(1)          # rotation for tiles [34, 72)
            stage_E(17, NPAIR)  # pairs 17-35
            for b in range(8, NBLK):
                stage_F(b)
    nc.finalize()
    return nc


_CACHE = {}


def _get_program(inputs):
    key = "prog"
    if key not in _CACHE:
        _CACHE[key] = build_program(
            np.asarray(inputs["Wc"], np.float32),
            np.asarray(inputs["bc"], np.float32),
            np.asarray(inputs["Woff"], np.float32),
            np.asarray(inputs["boff"], np.float32),
            np.asarray(inputs["Wwt"], np.float32),
            np.asarray(inputs["bwt"], np.float32),
        )
    return _CACHE[key]


def kernel(x, Wc, bc, Woff, boff, Wwt, bwt, _trace=False):
    from concourse.bass_utils import run_bass_kernel_spmd

    x = np.asarray(x, np.float32)
    b = x.shape[0]
    assert x.shape == (b, C, H, W) and b == 8

    nc = _get_program(dict(Wc=Wc, bc=bc, Woff=Woff, boff=boff, Wwt=Wwt, bwt=bwt))
    in_maps = [
        {"x": np.ascontiguousarray(x[i].reshape(C, N).astype(np.float16))}
        for i in range(b)
    ]
    res = run_bass_kernel_spmd(nc, in_maps, core_ids=list(range(b)), trace=_trace)
    _CACHE["last_results"] = res
    out = np.stack([res.results[i]["out"].reshape(O, H, W) for i in range(b)])
    return out.astype(np.float32)
